# revision 19
# baseline (speedup 1.0000x reference)
"""Causal multi-head self-attention (B=4, S=2048, D=1024, H=16, RoPE) on 8
Trainium2 NeuronCores.

Sharding (hardcoded): core c handles batch b = c//2 and head group g = c%2
(8 of the 16 heads).  Data parallel over B, tensor parallel over heads for
the Wq/Wk/Wv projections and over Wo rows/columns: each core computes the
attention output for its 8 heads, the two cores of a pair AllGather their
(normalized) per-head-pair attention chunks in 512-column slices, and each
core then computes a disjoint 512-wide column slice of the final Wo
projection for its batch, so the host only concatenates slices (no
host-side arithmetic).

Compute is bf16 on the tensor engine (f32 PSUM accumulation) throughout.

Schedule (v2): the kernel is software-pipelined across head pairs -- the
QKV projection + RoPE work for head pair hp+1 is emitted interleaved into
head pair hp's attention block loop (engine queues execute in program
order, so overlap must be explicit).  Per-slice softmax denominators are
reciprocated on the scalar engine via exp(-ln(den)) (same activation
table set as the softmax exp), keeping the slow DVE RECIPROCAL off the
critical path.  Diagonal score blocks use dedicated pre-zeroed prob tiles:
exp writes only the causally-valid column range and the mask multiply
touches only the 128-wide boundary strip.  Wo is computed per head-pair
PAIR (4 matmuls accumulated in one PSUM bank), halving the DVE
accumulation traffic; pair01 chunks run as tensor-engine filler during
hp2/hp3 attention, pair23 chunks trail the final AllGathers.
"""

import numpy as np

D_MODEL = 1024
NUM_HEADS = 16
ROPE_THETA = 10000.0
DH = D_MODEL // NUM_HEADS  # 64
SQT = 512  # sq tile width (= PSUM bank width in f32)


# ---------------------------------------------------------------------------
# Device kernel builder
# ---------------------------------------------------------------------------

def build_kernel(n_cores: int = 8, S: int = 2048):
    import concourse.bass as bass
    import concourse.mybir as mybir
    import concourse.tile as tile
    from concourse import bacc

    F32 = mybir.dt.float32
    BF16 = mybir.dt.bfloat16
    Exp = mybir.ActivationFunctionType.Exp
    Ln = mybir.ActivationFunctionType.Ln
    mult = mybir.AluOpType.mult
    add = mybir.AluOpType.add

    D = D_MODEL
    NC = D // 128          # 8 d-chunks
    NSB = S // 128         # s 128-blocks
    NSQ = S // SQT         # sq 512-tiles
    NHP = 4                # head pairs per core
    SWAP16 = list(range(16, 32)) + list(range(16))

    # Force exp/ln/copy onto the one table set that contains them all
    # (natural_log_exp_and_others); the default first-match selection would
    # otherwise reload tables on every exp<->ln alternation (~2.7us each).
    import concourse.hw_specs as _hw
    if not hasattr(bacc, "_orig_gat"):
        bacc._orig_gat = _hw.get_activation_tables

        def _gat_patched(arch):
            t = bacc._orig_gat(arch)
            strip = {mybir.ActivationFunctionType.Exp,
                     mybir.ActivationFunctionType.Ln,
                     mybir.ActivationFunctionType.Copy,
                     mybir.ActivationFunctionType.Identity}
            return {name: (fns if name == "natural_log_exp_and_others"
                           else (fns - strip))
                    for name, fns in t.items()}

        bacc.get_activation_tables = _gat_patched

    nc = bacc.Bacc("TRN2", target_bir_lowering=False, debug=False,
                   num_devices=n_cores)

    xT = nc.dram_tensor("xT", [128, NC, S], BF16, kind="ExternalInput")
    wqT = nc.dram_tensor("wqT", [128, NC, NHP, 128], BF16, kind="ExternalInput")
    wkT = nc.dram_tensor("wkT", [128, NC, NHP, 128], BF16, kind="ExternalInput")
    wvT = nc.dram_tensor("wvT", [128, NC, NHP, 128], BF16, kind="ExternalInput")
    woT = nc.dram_tensor("woT", [128, NC, SQT], BF16, kind="ExternalInput")
    cosT = nc.dram_tensor("cosT", [128, S], BF16, kind="ExternalInput")
    sinT = nc.dram_tensor("sinT", [128, S], BF16, kind="ExternalInput")
    masks = nc.dram_tensor("masks", [128, 2, 128], BF16, kind="ExternalInput")
    out = nc.dram_tensor("out", [S, SQT], F32, kind="ExternalOutput")

    groups = [[2 * i, 2 * i + 1] for i in range(n_cores // 2)]

    with tile.TileContext(nc) as tc:
        with (
            tc.tile_pool(name="const", bufs=1) as constp,
            tc.tile_pool(name="w", bufs=2) as wp,
            tc.tile_pool(name="qk", bufs=2) as qkp,
            tc.tile_pool(name="v", bufs=2) as vp,
            tc.tile_pool(name="probs", bufs=3) as probsp,
            tc.tile_pool(name="rope", bufs=2) as ropep,
            tc.tile_pool(name="attn", bufs=2) as attnp,
            tc.tile_pool(name="ag", bufs=8) as agp,
            tc.tile_pool(name="acc", bufs=1) as accp,
            tc.tile_pool(name="small", bufs=2) as smallp,
            tc.tile_pool(name="norm", bufs=2) as normp,
            tc.tile_pool(name="unn", bufs=1) as unnp,
            tc.tile_pool(name="psA", bufs=2, space="PSUM") as psA,
            tc.tile_pool(name="psQK", bufs=2, space="PSUM") as psQK,
            tc.tile_pool(name="psPV", bufs=2, space="PSUM") as psPV,
            tc.tile_pool(name="dram", bufs=16, space="DRAM") as dramp,
        ):
            # --- one-time loads (xt deferred until after hp0 weights) ----
            xt_sb = constp.tile([128, NC, S], BF16, tag="xt")
            wo_sb = constp.tile([128, NC, SQT], BF16, tag="wo")
            cos_sb = constp.tile([128, S], BF16, tag="cos")
            nc.sync.dma_start(cos_sb[:], cosT[:])
            sin_sb = constp.tile([128, S], BF16, tag="sin")
            nc.sync.dma_start(sin_sb[:], sinT[:])
            strip_sb = constp.tile([128, 2, 128], BF16, tag="strip")
            nc.sync.dma_start(strip_sb[:], masks[:])
            # dedicated diagonal-block prob tiles, pre-zeroed; exp only ever
            # writes the causally-reachable column range [128m, 512), so the
            # left zeros persist and PV can consume the full 512 columns.
            dzero = [constp.tile([128, 2, SQT], BF16, tag=f"dz{m}",
                                 name=f"dz{m}") for m in range(4)]
            for m in range(4):
                nc.gpsimd.memset(dzero[m][:], 0.0)

            out_acc = accp.tile([128, NSB, SQT], F32, tag="oacc")

            # --- per-head-pair state (python-side refs) -------------------
            state = {}

            def load_weights(hp):
                wq_sb = wp.tile([128, NC, 128], BF16, tag="wq")
                nc.sync.dma_start(wq_sb[:], wqT[:, :, hp, :])
                wk_sb = wp.tile([128, NC, 128], BF16, tag="wk")
                nc.sync.dma_start(wk_sb[:], wkT[:, :, hp, :])
                wv_sb = wp.tile([128, NC, 128], BF16, tag="wv")
                nc.sync.dma_start(wv_sb[:], wvT[:, :, hp, :])
                qT2 = qkp.tile([128, S], BF16, tag="qT")
                kT2 = qkp.tile([128, S], BF16, tag="kT")
                vaug = vp.tile([128, NSB, 130], BF16, tag="vaug")
                nc.gpsimd.memset(vaug[:, :, 64], 1.0)
                nc.gpsimd.memset(vaug[:, :, 129], 1.0)
                state[hp] = dict(wq=wq_sb, wk=wk_sb, wv=wv_sb,
                                 qT=qT2, kT=kT2, vaug=vaug)

            def qk_unit(hp, which, j):
                # projection + RoPE for one 512-wide j tile of q or k
                st = state[hp]
                w_sb = st['wq'] if which == 'q' else st['wk']
                dst = st['qT'] if which == 'q' else st['kT']
                jsl = bass.ts(j, SQT)
                ps = psA.tile([128, SQT], F32, tag="psA")
                for c in range(NC):
                    nc.tensor.matmul(ps[:], w_sb[:, c, :], xt_sb[:, c, jsl],
                                     start=(c == 0), stop=(c == NC - 1))
                # rotate: dst = q*cos + shuffle16(q)*sin_signed.  All DVE
                # except the final add (gpsimd) -- keeping the ACT queue
                # exp-only and the gpsimd queue short avoids head-of-line
                # stalls in the in-order engine queues.
                qsb = smallp.tile([128, SQT], BF16, tag="qsb")
                nc.vector.tensor_copy(qsb[:], ps[:])
                t1 = ropep.tile([128, SQT], BF16, tag="t1")
                nc.vector.tensor_tensor(
                    out=t1[:], in0=qsb[:], in1=cos_sb[:, jsl], op=mult)
                sh = ropep.tile([128, SQT], BF16, tag="sh")
                nc.vector.stream_shuffle(sh[:], qsb[:], SWAP16)
                t2 = ropep.tile([128, SQT], BF16, tag="t2")
                nc.gpsimd.tensor_tensor(
                    out=t2[:], in0=sh[:], in1=sin_sb[:, jsl], op=mult)
                nc.gpsimd.tensor_tensor(
                    out=dst[:, jsl], in0=t1[:], in1=t2[:], op=add)

            def v_unit(hp, sb):
                # v for one 128-row s block, computed directly in [s, dv]
                # layout (x block stationary, Wv moving) -- no transposes,
                # and the PE never waits on a DVE intermediate
                st = state[hp]
                wv_sb, vaug = st['wv'], st['vaug']
                ps = psA.tile([128, 128], F32, tag="psA", name="psv")
                for c in range(NC):
                    nc.tensor.matmul(ps[:], xt_sb[:, c, bass.ts(sb, 128)],
                                     wv_sb[:, c, :],
                                     start=(c == 0), stop=(c == NC - 1))
                nc.vector.tensor_copy(vaug[:, sb, 0:64], ps[:, 0:64])
                nc.vector.tensor_copy(vaug[:, sb, 65:129], ps[:, 64:128])

            def proj_units(hp):
                units = []
                for j in range(NSQ):
                    units.append(lambda hp=hp, j=j: qk_unit(hp, 'q', j))
                    units.append(lambda hp=hp, j=j: qk_unit(hp, 'k', j))
                for sb in range(NSB):
                    units.append(lambda hp=hp, sb=sb: v_unit(hp, sb))
                return units

            # --- Wo: 4 matmuls (2 head pairs x 2 groups) per PSUM bank ---
            agt = {}
            agd = {}

            def load_ag(keys):
                # batched SBUF reload of AllGather results, emitted well
                # after the AGs completed so the in-order Sync queue never
                # blocks waiting on a collective
                for a, j in keys:
                    ag01 = agp.tile([128, 2, SQT], BF16, tag="ag01",
                                    name="ag01")
                    nc.sync.dma_start(ag01[:, 0, :], agd[(a, j)][0])
                    nc.sync.dma_start(ag01[:, 1, :], agd[(a, j)][1])
                    agt[(a, j)] = ag01

            def emit_wo_quarter(a, b, j, t, final):
                sb = (SQT // 128) * j + t
                tsl = bass.ts(t, 128)
                ps = psA.tile([128, SQT], F32, tag="psA")
                nc.tensor.matmul(ps[:], agt[(a, j)][:, 0, tsl],
                                 wo_sb[:, a, :], start=True, stop=False)
                nc.tensor.matmul(ps[:], agt[(a, j)][:, 1, tsl],
                                 wo_sb[:, NC // 2 + a, :],
                                 start=False, stop=False)
                nc.tensor.matmul(ps[:], agt[(b, j)][:, 0, tsl],
                                 wo_sb[:, b, :], start=False, stop=False)
                nc.tensor.matmul(ps[:], agt[(b, j)][:, 1, tsl],
                                 wo_sb[:, NC // 2 + b, :],
                                 start=False, stop=True)
                if not final:
                    nc.vector.tensor_copy(out_acc[:, sb, :], ps[:])
                else:
                    nc.vector.tensor_tensor(
                        out=out_acc[:, sb, :], in0=out_acc[:, sb, :],
                        in1=ps[:], op=add)
                    nc.sync.dma_start(out[bass.ts(sb, 128), :],
                                      out_acc[:, sb, :])

            # --- prologue: weights + projections for hp 0.  The xt DMAs
            # are j-major so the first projection tiles start while the
            # rest of x is still in flight. ------------------------------
            load_weights(0)
            for j in range(NSQ):
                jsl = bass.ts(j, SQT)
                for c in range(NC):
                    nc.sync.dma_start(xt_sb[:, c, jsl], xT[:, c, jsl])
            nc.sync.dma_start(wo_sb[:], woT[:])
            for u in proj_units(0):
                u()

            # --- main pipelined loop -------------------------------------
            pending_tail = [None]

            for hp in range(NHP):
                st = state[hp]
                qT2, kT2, vaug = st['qT'], st['kT'], st['vaug']
                fillers = []
                if hp + 1 < NHP:
                    load_weights(hp + 1)
                    fillers += proj_units(hp + 1)
                if hp == 2:
                    # pair01 Wo for j=0,1: flush hp1's deferred tail (its
                    # AllGather trigger) first, then batch-load the results
                    if pending_tail[0] is not None:
                        pending_tail[0]()
                        pending_tail[0] = None
                    load_ag([(0, j) for j in range(NSQ)]
                            + [(1, j) for j in range(NSQ - 1)])
                    fillers += [
                        lambda j=j, t=t: emit_wo_quarter(0, 1, j, t, False)
                        for j in range(2) for t in range(SQT // 128)]
                if hp == 3:
                    fillers += [
                        lambda j=j, t=t: emit_wo_quarter(0, 1, j, t, False)
                        for j in range(2, NSQ) for t in range(SQT // 128)]
                block_ctr = 0

                attnT2 = attnp.tile([128, S], BF16, tag="attnT")
                unnorm = unnp.tile([128, S], BF16, tag="unnorm")
                for j in range(NSQ):
                    if hp == 2 and j == 1:
                        load_ag([(1, NSQ - 1)])
                    if hp == 3 and j >= 2:
                        # pair23 Wo for slice j-2: its AllGather was
                        # triggered two slices ago, safely complete
                        load_ag([(2, j - 2), (3, j - 2)])
                        fillers.extend(
                            lambda t=t, jj=j - 2:
                            emit_wo_quarter(2, 3, jj, t, True)
                            for t in range(SQT // 128))
                    jsl = bass.ts(j, SQT)
                    pv0 = psPV.tile([128, SQT], F32, tag="pv")
                    pv1 = psPV.tile([128, SQT], F32, tag="pv")
                    n_sk = (SQT // 128) * j + 4

                    def emit_qk(i, j=j, jsl=jsl):
                        # scores for block i (both heads, concurrent PE row
                        # tiles); on diagonal blocks only columns >= 128m
                        # are causally reachable
                        m = i - (SQT // 128) * j
                        qk2 = psQK.tile([128, 2, SQT], F32, tag="qk",
                                        name="qk2")
                        if m < 0:
                            for h in range(2):
                                nc.tensor.matmul(
                                    qk2[:, h, :],
                                    kT2[64 * h:64 * h + 64, bass.ts(i, 128)],
                                    qT2[64 * h:64 * h + 64, jsl],
                                    start=True, stop=True)
                        else:
                            off = 128 * m
                            for h in range(2):
                                nc.tensor.matmul(
                                    qk2[:, h, off:],
                                    kT2[64 * h:64 * h + 64, bass.ts(i, 128)],
                                    qT2[64 * h:64 * h + 64,
                                        j * SQT + off:(j + 1) * SQT],
                                    start=True, stop=True)
                        return qk2, m

                    # two-block QK lookahead keeps the scalar engine's exp
                    # stream saturated: the PE queue always holds the next
                    # QK before the (exp-dependent) PV of the current block
                    qks = {0: emit_qk(0), 1: emit_qk(1)}
                    for i in range(n_sk):
                        qk2, m = qks.pop(i)
                        if m < 0:
                            pr2 = probsp.tile([128, 2, SQT], BF16,
                                              tag="probs", name="pr2")
                            nc.scalar.activation(pr2[:], qk2[:], Exp,
                                                 scale=0.125)
                            src = pr2
                        else:
                            off = 128 * m
                            dm = dzero[m]
                            nc.scalar.activation(dm[:, :, off:],
                                                 qk2[:, :, off:], Exp,
                                                 scale=0.125)
                            # mask only the 128-wide boundary strip
                            nc.vector.tensor_tensor(
                                out=dm[:, :, off:off + 128],
                                in0=dm[:, :, off:off + 128],
                                in1=strip_sb[:], op=mult)
                            src = dm
                        if i == 1 and pending_tail[0] is not None:
                            pending_tail[0]()
                            pending_tail[0] = None
                        if i + 2 < n_sk:
                            qks[i + 2] = emit_qk(i + 2)
                        if fillers:
                            fillers.pop(0)()
                        block_ctr += 1
                        for h, pv in ((0, pv0), (1, pv1)):
                            nc.tensor.matmul(
                                pv[0:65, :], vaug[:, i, 65 * h:65 * h + 65],
                                src[:, h, :],
                                start=(i == 0), stop=(i == n_sk - 1))

                    # ---- stage PV results now (frees the PV PSUM banks) --
                    den_a = normp.tile([1, SQT], F32, tag="den_a")
                    den_b = normp.tile([1, SQT], F32, tag="den_b")
                    # (dens stay f32: ln wants the full exponent range)
                    nc.vector.tensor_copy(unnorm[0:64, jsl], pv0[0:64, :])
                    nc.vector.tensor_copy(unnorm[64:128, jsl], pv1[0:64, :])
                    nc.vector.tensor_copy(den_a[:], pv0[64:65, :])
                    nc.vector.tensor_copy(den_b[:], pv1[64:65, :])

                    def tail(hp=hp, j=j, jsl=jsl, den_a=den_a, den_b=den_b,
                             attnT2=attnT2, unnorm=unnorm):
                        # 1/den on the scalar engine: exp(-ln(den)) stays
                        # within the natural_log_exp_and_others table set.
                        # Separate per-head tiles (partition 0) keep the
                        # ACT->gpsimd->DVE tail chain linear.
                        rec_a = normp.tile([1, SQT], BF16, tag="rec_a")
                        rec_b = normp.tile([1, SQT], BF16, tag="rec_b")
                        lnt = normp.tile([1, SQT], F32, tag="lnt")
                        nc.scalar.activation(lnt[:], den_a[:], Ln)
                        nc.scalar.activation(rec_a[:], lnt[:], Exp,
                                             scale=-1.0)
                        lnt2 = normp.tile([1, SQT], F32, tag="lnt2")
                        nc.scalar.activation(lnt2[:], den_b[:], Ln)
                        nc.scalar.activation(rec_b[:], lnt2[:], Exp,
                                             scale=-1.0)
                        reca = normp.tile([128, SQT], BF16, tag="reca")
                        nc.gpsimd.partition_broadcast(reca[:], rec_a[:])
                        recb = normp.tile([128, SQT], BF16, tag="recb")
                        nc.gpsimd.partition_broadcast(recb[:], rec_b[:])
                        nc.vector.tensor_tensor(
                            out=attnT2[0:64, jsl], in0=unnorm[0:64, jsl],
                            in1=reca[0:64, :], op=mult)
                        nc.vector.tensor_tensor(
                            out=attnT2[64:128, jsl], in0=unnorm[64:128, jsl],
                            in1=recb[64:128, :], op=mult)

                        ag_in = dramp.tile([128, SQT], BF16, tag="ag_in")
                        nc.sync.dma_start(ag_in[:], attnT2[:, jsl])
                        ag_out = dramp.tile([2, 128, SQT], BF16, tag="ag_out")
                        nc.gpsimd.collective_compute(
                            "AllGather", mybir.AluOpType.bypass,
                            ins=[ag_in[:].opt()], outs=[ag_out[:].opt()],
                            replica_groups=groups)
                        agd[(hp, j)] = ag_out

                    # normalize/AllGather deferred into the next slice's
                    # block loop so the denorm ACT ops never gap the exps
                    pending_tail[0] = tail

            if pending_tail[0] is not None:
                pending_tail[0]()
                pending_tail[0] = None

            # --- drain: remaining pair23 Wo chunks -----------------------
            load_ag([(2, j) for j in range(NSQ - 2, NSQ)]
                    + [(3, j) for j in range(NSQ - 2, NSQ)])
            for j in range(NSQ - 2, NSQ):
                for t in range(SQT // 128):
                    emit_wo_quarter(2, 3, j, t, final=True)

    nc.compile()
    return nc


# ---------------------------------------------------------------------------
# Host-side sharding / unsharding
# ---------------------------------------------------------------------------

def _host_inputs(x, Wq, Wk, Wv, Wo, token_positions, n_cores, S):
    import ml_dtypes
    bf16 = ml_dtypes.bfloat16
    D = D_MODEL
    NC = D // 128
    NHP = 4

    # rope tables.  Partition layout within each head (64 partitions):
    # [e0..e15, o0..o15, e16..e31, o16..o31] -- the rotation partner sits
    # 16 partitions away inside the same 32-group, so the kernel's
    # stream_shuffle (a per-32-group lane shuffle) can realize the swap.
    pos = np.asarray(token_positions).astype(np.float32)  # (S,)
    i32 = np.arange(32, dtype=np.float32)
    inv_freq = ROPE_THETA ** (-i32 / 32.0)
    ang = pos[None, :] * inv_freq[:, None]              # (32, S)
    p = np.arange(128)
    pp = p % 64
    g, o = pp // 32, pp % 32
    freq_idx = 16 * g + (o % 16)                        # (128,)
    sign = np.where(o % 32 < 16, -1.0, 1.0)             # even slots: -sin
    cosT = np.cos(ang[freq_idx, :]).astype(bf16)        # (128, S)
    sinT = (sign[:, None] * np.sin(ang[freq_idx, :])).astype(bf16)

    # causal boundary strip mask (same pattern for every diagonal block
    # offset), duplicated for the two heads: valid iff pcol <= f
    pcol = np.arange(128)[:, None]
    f = np.arange(128)[None, :]
    strip = (pcol <= f)
    masks = np.tile(strip, (1, 2)).astype(bf16)         # (128, 256)

    # de-interleaving row permutation for q/k (see rope table comment)
    def qk_rows(grp):
        rows = []
        for h in range(8 * grp, 8 * grp + 8):
            rows += [h * DH + 2 * i for i in range(16)]
            rows += [h * DH + 2 * i + 1 for i in range(16)]
            rows += [h * DH + 2 * i for i in range(16, 32)]
            rows += [h * DH + 2 * i + 1 for i in range(16, 32)]
        return rows

    def wqk_layout(W, grp):
        # (D, 512) -> [128, NC, NHP, 128]
        t = W[qk_rows(grp), :].T.astype(bf16)
        return np.ascontiguousarray(
            t.reshape(NC, 128, NHP, 128).transpose(1, 0, 2, 3))

    def wv_layout(W, grp):
        t = W[512 * grp:512 * grp + 512, :].T.astype(bf16)
        return np.ascontiguousarray(
            t.reshape(NC, 128, NHP, 128).transpose(1, 0, 2, 3))

    def wo_layout(W, grp):
        t = W.T[:, 512 * grp:512 * grp + 512].astype(bf16)  # (D, 512)
        return np.ascontiguousarray(t.reshape(NC, 128, SQT).transpose(1, 0, 2))

    in_maps = []
    for c in range(n_cores):
        b, grp = c // 2, c % 2
        xb = np.ascontiguousarray(x[b].T).astype(bf16)  # (D, S)
        in_maps.append({
            "xT": np.ascontiguousarray(
                xb.reshape(NC, 128, S).transpose(1, 0, 2)),
            "wqT": wqk_layout(Wq, grp),
            "wkT": wqk_layout(Wk, grp),
            "wvT": wv_layout(Wv, grp),
            "woT": wo_layout(Wo, grp),
            "cosT": cosT,
            "sinT": sinT,
            "masks": masks.reshape(128, 2, 128),
        })
    return in_maps


def _assemble(results, n_cores, S):
    B = n_cores // 2
    full = np.empty((B, S, D_MODEL), dtype=np.float32)
    for c in range(n_cores):
        b, grp = c // 2, c % 2
        full[b, :, 512 * grp:512 * grp + 512] = results[c]["out"]
    return full


# ---------------------------------------------------------------------------
# Entry point
# ---------------------------------------------------------------------------

_NC_CACHE = {}


def _get_nc(n_cores, S):
    key = (n_cores, S)
    if key not in _NC_CACHE:
        _NC_CACHE[key] = build_kernel(n_cores, S)
    return _NC_CACHE[key]


def kernel(x, Wq, Wk, Wv, Wo, token_positions, _trace=False, _tmpdir=None):
    from concourse.bass_utils import run_bass_kernel_spmd

    x = np.asarray(x)
    B, S, D = x.shape
    n_cores = 2 * B
    nc = _get_nc(n_cores, S)
    in_maps = _host_inputs(np.asarray(x), np.asarray(Wq), np.asarray(Wk),
                           np.asarray(Wv), np.asarray(Wo),
                           np.asarray(token_positions), n_cores, S)
    res = run_bass_kernel_spmd(nc, in_maps, core_ids=list(range(n_cores)),
                               trace=_trace, tmpdir=_tmpdir)
    out = _assemble(res.results, n_cores, S)
    if _trace:
        return out, res
    return out


# revision 21
# speedup vs baseline: 1.0414x; 1.0414x over previous
"""Causal multi-head self-attention (B=4, S=2048, D=1024, H=16, RoPE) on 8
Trainium2 NeuronCores.

Sharding (hardcoded): core c handles batch b = c//2 and head group g = c%2
(8 of the 16 heads).  Data parallel over B, tensor parallel over heads for
the Wq/Wk/Wv projections and over Wo rows/columns: each core computes the
attention output for its 8 heads, the two cores of a pair AllGather their
(normalized) per-head-pair attention chunks in 512-column slices, and each
core then computes a disjoint 512-wide column slice of the final Wo
projection for its batch, so the host only concatenates slices (no
host-side arithmetic).

Compute is bf16 on the tensor engine (f32 PSUM accumulation) throughout.

Schedule (v2): the kernel is software-pipelined across head pairs -- the
QKV projection + RoPE work for head pair hp+1 is emitted interleaved into
head pair hp's attention block loop (engine queues execute in program
order, so overlap must be explicit).  Per-slice softmax denominators are
reciprocated on the scalar engine via exp(-ln(den)) (same activation
table set as the softmax exp), keeping the slow DVE RECIPROCAL off the
critical path.  Diagonal score blocks use dedicated pre-zeroed prob tiles:
exp writes only the causally-valid column range and the mask multiply
touches only the 128-wide boundary strip.  Wo is computed per head-pair
PAIR (4 matmuls accumulated in one PSUM bank), halving the DVE
accumulation traffic; pair01 chunks run as tensor-engine filler during
hp2/hp3 attention, pair23 chunks trail the final AllGathers.
"""

import numpy as np

D_MODEL = 1024
NUM_HEADS = 16
ROPE_THETA = 10000.0
DH = D_MODEL // NUM_HEADS  # 64
SQT = 512  # sq tile width (= PSUM bank width in f32)


# ---------------------------------------------------------------------------
# Device kernel builder
# ---------------------------------------------------------------------------

def build_kernel(n_cores: int = 8, S: int = 2048):
    import concourse.bass as bass
    import concourse.mybir as mybir
    import concourse.tile as tile
    from concourse import bacc

    F32 = mybir.dt.float32
    BF16 = mybir.dt.bfloat16
    Exp = mybir.ActivationFunctionType.Exp
    Ln = mybir.ActivationFunctionType.Ln
    mult = mybir.AluOpType.mult
    add = mybir.AluOpType.add

    D = D_MODEL
    NC = D // 128          # 8 d-chunks
    NSB = S // 128         # s 128-blocks
    NSQ = S // SQT         # sq 512-tiles
    NHP = 4                # head pairs per core
    SWAP16 = list(range(16, 32)) + list(range(16))

    # Force exp/ln/copy onto the one table set that contains them all
    # (natural_log_exp_and_others); the default first-match selection would
    # otherwise reload tables on every exp<->ln alternation (~2.7us each).
    import concourse.hw_specs as _hw
    if not hasattr(bacc, "_orig_gat"):
        bacc._orig_gat = _hw.get_activation_tables

        def _gat_patched(arch):
            t = bacc._orig_gat(arch)
            strip = {mybir.ActivationFunctionType.Exp,
                     mybir.ActivationFunctionType.Ln,
                     mybir.ActivationFunctionType.Copy,
                     mybir.ActivationFunctionType.Identity}
            return {name: (fns if name == "natural_log_exp_and_others"
                           else (fns - strip))
                    for name, fns in t.items()}

        bacc.get_activation_tables = _gat_patched

    nc = bacc.Bacc("TRN2", target_bir_lowering=False, debug=False,
                   num_devices=n_cores)

    xT = nc.dram_tensor("xT", [128, NC, S], BF16, kind="ExternalInput")
    wqT = nc.dram_tensor("wqT", [128, NC, NHP, 128], BF16, kind="ExternalInput")
    wkT = nc.dram_tensor("wkT", [128, NC, NHP, 128], BF16, kind="ExternalInput")
    wvT = nc.dram_tensor("wvT", [128, NC, NHP, 128], BF16, kind="ExternalInput")
    woT = nc.dram_tensor("woT", [128, NC, SQT], BF16, kind="ExternalInput")
    cosT = nc.dram_tensor("cosT", [128, S], BF16, kind="ExternalInput")
    sinT = nc.dram_tensor("sinT", [128, S], BF16, kind="ExternalInput")
    masks = nc.dram_tensor("masks", [128, 2, 128], BF16, kind="ExternalInput")
    out = nc.dram_tensor("out", [S, SQT], F32, kind="ExternalOutput")

    groups = [[2 * i, 2 * i + 1] for i in range(n_cores // 2)]

    with tile.TileContext(nc) as tc:
        with (
            tc.tile_pool(name="const", bufs=1) as constp,
            tc.tile_pool(name="w", bufs=2) as wp,
            tc.tile_pool(name="qk", bufs=2) as qkp,
            tc.tile_pool(name="v", bufs=2) as vp,
            tc.tile_pool(name="probs", bufs=3) as probsp,
            tc.tile_pool(name="rope", bufs=2) as ropep,
            tc.tile_pool(name="attn", bufs=2) as attnp,
            tc.tile_pool(name="ag", bufs=8) as agp,
            tc.tile_pool(name="acc", bufs=1) as accp,
            tc.tile_pool(name="small", bufs=2) as smallp,
            tc.tile_pool(name="norm", bufs=2) as normp,
            tc.tile_pool(name="unn", bufs=1) as unnp,
            tc.tile_pool(name="psA", bufs=2, space="PSUM") as psA,
            tc.tile_pool(name="psQK", bufs=2, space="PSUM") as psQK,
            tc.tile_pool(name="psPV", bufs=2, space="PSUM") as psPV,
            tc.tile_pool(name="dram", bufs=16, space="DRAM") as dramp,
        ):
            # --- one-time loads (xt deferred until after hp0 weights) ----
            xt_sb = constp.tile([128, NC, S], BF16, tag="xt")
            wo_sb = constp.tile([128, NC, SQT], BF16, tag="wo")
            cos_sb = constp.tile([128, S], BF16, tag="cos")
            nc.sync.dma_start(cos_sb[:], cosT[:])
            sin_sb = constp.tile([128, S], BF16, tag="sin")
            nc.sync.dma_start(sin_sb[:], sinT[:])
            strip_sb = constp.tile([128, 2, 128], BF16, tag="strip")
            nc.sync.dma_start(strip_sb[:], masks[:])
            # dedicated diagonal-block prob tiles, pre-zeroed; exp only ever
            # writes the causally-reachable column range [128m, 512), so the
            # left zeros persist and PV can consume the full 512 columns.
            dzero = [constp.tile([128, 2, SQT], BF16, tag=f"dz{m}",
                                 name=f"dz{m}") for m in range(4)]
            for m in range(4):
                nc.gpsimd.memset(dzero[m][:], 0.0)

            out_acc = accp.tile([128, NSB, SQT], F32, tag="oacc")

            # --- per-head-pair state (python-side refs) -------------------
            state = {}

            def load_weights(hp):
                wq_sb = wp.tile([128, NC, 128], BF16, tag="wq")
                nc.sync.dma_start(wq_sb[:], wqT[:, :, hp, :])
                wk_sb = wp.tile([128, NC, 128], BF16, tag="wk")
                nc.sync.dma_start(wk_sb[:], wkT[:, :, hp, :])
                wv_sb = wp.tile([128, NC, 128], BF16, tag="wv")
                nc.sync.dma_start(wv_sb[:], wvT[:, :, hp, :])
                qT2 = qkp.tile([128, S], BF16, tag="qT")
                kT2 = qkp.tile([128, S], BF16, tag="kT")
                vaug = vp.tile([128, NSB, 130], BF16, tag="vaug")
                nc.gpsimd.memset(vaug[:, :, 64], 1.0)
                nc.gpsimd.memset(vaug[:, :, 129], 1.0)
                state[hp] = dict(wq=wq_sb, wk=wk_sb, wv=wv_sb,
                                 qT=qT2, kT=kT2, vaug=vaug)

            def qk_unit(hp, which, j):
                # projection + RoPE for one 512-wide j tile of q or k
                st = state[hp]
                w_sb = st['wq'] if which == 'q' else st['wk']
                dst = st['qT'] if which == 'q' else st['kT']
                jsl = bass.ts(j, SQT)
                ps = psA.tile([128, SQT], F32, tag="psA")
                for c in range(NC):
                    nc.tensor.matmul(ps[:], w_sb[:, c, :], xt_sb[:, c, jsl],
                                     start=(c == 0), stop=(c == NC - 1))
                # rotate: dst = q*cos + shuffle16(q)*sin_signed.  All DVE
                # except the final add (gpsimd) -- keeping the ACT queue
                # exp-only and the gpsimd queue short avoids head-of-line
                # stalls in the in-order engine queues.
                qsb = smallp.tile([128, SQT], BF16, tag="qsb")
                nc.vector.tensor_copy(qsb[:], ps[:])
                t1 = ropep.tile([128, SQT], BF16, tag="t1")
                nc.vector.tensor_tensor(
                    out=t1[:], in0=qsb[:], in1=cos_sb[:, jsl], op=mult)
                sh = ropep.tile([128, SQT], BF16, tag="sh")
                nc.vector.stream_shuffle(sh[:], qsb[:], SWAP16)
                t2 = ropep.tile([128, SQT], BF16, tag="t2")
                nc.vector.tensor_tensor(
                    out=t2[:], in0=sh[:], in1=sin_sb[:, jsl], op=mult)
                nc.gpsimd.tensor_tensor(
                    out=dst[:, jsl], in0=t1[:], in1=t2[:], op=add)

            def v_unit(hp, sb):
                # v for one 128-row s block, computed directly in [s, dv]
                # layout (x block stationary, Wv moving) -- no transposes,
                # and the PE never waits on a DVE intermediate
                st = state[hp]
                wv_sb, vaug = st['wv'], st['vaug']
                ps = psA.tile([128, 128], F32, tag="psA", name="psv")
                for c in range(NC):
                    nc.tensor.matmul(ps[:], xt_sb[:, c, bass.ts(sb, 128)],
                                     wv_sb[:, c, :],
                                     start=(c == 0), stop=(c == NC - 1))
                nc.vector.tensor_copy(vaug[:, sb, 0:64], ps[:, 0:64])
                nc.vector.tensor_copy(vaug[:, sb, 65:129], ps[:, 64:128])

            def proj_units(hp):
                units = []
                for j in range(NSQ):
                    units.append(lambda hp=hp, j=j: qk_unit(hp, 'q', j))
                    units.append(lambda hp=hp, j=j: qk_unit(hp, 'k', j))
                for sb in range(NSB):
                    units.append(lambda hp=hp, sb=sb: v_unit(hp, sb))
                return units

            # --- Wo: 4 matmuls (2 head pairs x 2 groups) per PSUM bank ---
            agt = {}
            agd = {}

            def load_ag(keys):
                # batched SBUF reload of AllGather results, emitted well
                # after the AGs completed so the in-order Sync queue never
                # blocks waiting on a collective
                for a, j in keys:
                    ag01 = agp.tile([128, 2, SQT], BF16, tag="ag01",
                                    name="ag01")
                    nc.sync.dma_start(ag01[:, 0, :], agd[(a, j)][0])
                    nc.sync.dma_start(ag01[:, 1, :], agd[(a, j)][1])
                    agt[(a, j)] = ag01

            def emit_wo_quarter(a, b, j, t, final):
                sb = (SQT // 128) * j + t
                tsl = bass.ts(t, 128)
                ps = psA.tile([128, SQT], F32, tag="psA")
                nc.tensor.matmul(ps[:], agt[(a, j)][:, 0, tsl],
                                 wo_sb[:, a, :], start=True, stop=False)
                nc.tensor.matmul(ps[:], agt[(a, j)][:, 1, tsl],
                                 wo_sb[:, NC // 2 + a, :],
                                 start=False, stop=False)
                nc.tensor.matmul(ps[:], agt[(b, j)][:, 0, tsl],
                                 wo_sb[:, b, :], start=False, stop=False)
                nc.tensor.matmul(ps[:], agt[(b, j)][:, 1, tsl],
                                 wo_sb[:, NC // 2 + b, :],
                                 start=False, stop=True)
                if not final:
                    nc.vector.tensor_copy(out_acc[:, sb, :], ps[:])
                else:
                    nc.vector.tensor_tensor(
                        out=out_acc[:, sb, :], in0=out_acc[:, sb, :],
                        in1=ps[:], op=add)
                    nc.sync.dma_start(out[bass.ts(sb, 128), :],
                                      out_acc[:, sb, :])

            # --- prologue: weights + projections for hp 0.  The xt DMAs
            # are j-major so the first projection tiles start while the
            # rest of x is still in flight. ------------------------------
            load_weights(0)
            for j in range(NSQ):
                jsl = bass.ts(j, SQT)
                for c in range(NC):
                    nc.sync.dma_start(xt_sb[:, c, jsl], xT[:, c, jsl])
            nc.sync.dma_start(wo_sb[:], woT[:])
            for u in proj_units(0):
                u()

            # --- main pipelined loop: one flat block sequence across all
            # head pairs and slices, with a global two-block QK lookahead.
            # The lookahead crosses slice/hp boundaries so the next QK is
            # always in the PE queue ahead of the filler backlog, keeping
            # the exp stream fed. -----------------------------------------
            pending_tail = [None]
            blocks = [(hp, j, i) for hp in range(NHP) for j in range(NSQ)
                      for i in range((SQT // 128) * j + 4)]
            qks = {}
            pvs = {}
            fillers = []

            def emit_qk(hp, j, i):
                st = state[hp]
                qT2, kT2 = st['qT'], st['kT']
                m = i - (SQT // 128) * j
                qk2 = psQK.tile([128, 2, SQT], F32, tag="qk", name="qk2")
                if m < 0:
                    for h in range(2):
                        nc.tensor.matmul(
                            qk2[:, h, :],
                            kT2[64 * h:64 * h + 64, bass.ts(i, 128)],
                            qT2[64 * h:64 * h + 64, bass.ts(j, SQT)],
                            start=True, stop=True)
                else:
                    off = 128 * m
                    for h in range(2):
                        nc.tensor.matmul(
                            qk2[:, h, off:],
                            kT2[64 * h:64 * h + 64, bass.ts(i, 128)],
                            qT2[64 * h:64 * h + 64,
                                j * SQT + off:(j + 1) * SQT],
                            start=True, stop=True)
                return qk2, m

            qks[blocks[0]] = emit_qk(*blocks[0])
            qks[blocks[1]] = emit_qk(*blocks[1])

            for g, (hp, j, i) in enumerate(blocks):
                st = state[hp]
                n_sk = (SQT // 128) * j + 4
                if j == 0 and i == 0:
                    # --- head-pair bookkeeping -----------------------
                    if hp + 1 < NHP:
                        load_weights(hp + 1)
                        fillers += proj_units(hp + 1)
                    if hp == 2:
                        # flush hp1's deferred tail (its AllGather
                        # trigger), then batch-load pair01 results
                        if pending_tail[0] is not None:
                            pending_tail[0]()
                            pending_tail[0] = None
                        load_ag([(0, jj) for jj in range(NSQ)]
                                + [(1, jj) for jj in range(NSQ - 1)])
                        fillers += [
                            lambda jj=jj, t=t:
                            emit_wo_quarter(0, 1, jj, t, False)
                            for jj in range(2) for t in range(SQT // 128)]
                    if hp == 3:
                        fillers += [
                            lambda jj=jj, t=t:
                            emit_wo_quarter(0, 1, jj, t, False)
                            for jj in range(2, NSQ)
                            for t in range(SQT // 128)]
                    st['attnT'] = attnp.tile([128, S], BF16, tag="attnT",
                                             name="attnT")
                    st['unnorm'] = unnp.tile([128, S], BF16, tag="unnorm",
                                             name="unnorm")
                if i == 0:
                    if hp == 2 and j == 1:
                        load_ag([(1, NSQ - 1)])
                    if hp == 3 and j >= 2:
                        # pair23 Wo for slice j-2: its AllGather was
                        # triggered two slices ago, safely complete
                        load_ag([(2, j - 2), (3, j - 2)])
                        fillers.extend(
                            lambda t=t, jj=j - 2:
                            emit_wo_quarter(2, 3, jj, t, True)
                            for t in range(SQT // 128))
                    pvs[(hp, j)] = (
                        psPV.tile([128, SQT], F32, tag="pv", name="pv0"),
                        psPV.tile([128, SQT], F32, tag="pv", name="pv1"))
                pv0, pv1 = pvs[(hp, j)]
                jsl = bass.ts(j, SQT)

                qk2, m = qks.pop((hp, j, i))
                if m < 0:
                    pr2 = probsp.tile([128, 2, SQT], BF16,
                                      tag="probs", name="pr2")
                    nc.scalar.activation(pr2[:], qk2[:], Exp, scale=0.125)
                    src = pr2
                else:
                    off = 128 * m
                    dm = dzero[m]
                    nc.scalar.activation(dm[:, :, off:],
                                         qk2[:, :, off:], Exp, scale=0.125)
                    # mask only the 128-wide boundary strip
                    nc.vector.tensor_tensor(
                        out=dm[:, :, off:off + 128],
                        in0=dm[:, :, off:off + 128],
                        in1=strip_sb[:], op=mult)
                    src = dm
                if i == 1 and pending_tail[0] is not None:
                    pending_tail[0]()
                    pending_tail[0] = None
                if g + 2 < len(blocks):
                    qks[blocks[g + 2]] = emit_qk(*blocks[g + 2])
                if fillers:
                    fillers.pop(0)()
                vaug = st['vaug']
                for h, pv in ((0, pv0), (1, pv1)):
                    nc.tensor.matmul(
                        pv[0:65, :], vaug[:, i, 65 * h:65 * h + 65],
                        src[:, h, :],
                        start=(i == 0), stop=(i == n_sk - 1))

                if i == n_sk - 1:
                    # ---- stage PV results now (frees the PV banks) ------
                    attnT2, unnorm = st['attnT'], st['unnorm']
                    den_a = normp.tile([1, SQT], F32, tag="den_a")
                    den_b = normp.tile([1, SQT], F32, tag="den_b")
                    nc.vector.tensor_copy(unnorm[0:64, jsl], pv0[0:64, :])
                    nc.vector.tensor_copy(unnorm[64:128, jsl], pv1[0:64, :])
                    nc.vector.tensor_copy(den_a[:], pv0[64:65, :])
                    nc.vector.tensor_copy(den_b[:], pv1[64:65, :])
                    pvs.pop((hp, j))

                    def tail(hp=hp, j=j, jsl=jsl, den_a=den_a, den_b=den_b,
                             attnT2=attnT2, unnorm=unnorm):
                        # 1/den on the scalar engine: exp(-ln(den)) stays
                        # within the natural_log_exp_and_others table set
                        rec_a = normp.tile([1, SQT], BF16, tag="rec_a")
                        rec_b = normp.tile([1, SQT], BF16, tag="rec_b")
                        lnt = normp.tile([1, SQT], F32, tag="lnt")
                        nc.scalar.activation(lnt[:], den_a[:], Ln)
                        nc.scalar.activation(rec_a[:], lnt[:], Exp,
                                             scale=-1.0)
                        lnt2 = normp.tile([1, SQT], F32, tag="lnt2")
                        nc.scalar.activation(lnt2[:], den_b[:], Ln)
                        nc.scalar.activation(rec_b[:], lnt2[:], Exp,
                                             scale=-1.0)
                        reca = normp.tile([128, SQT], BF16, tag="reca")
                        nc.gpsimd.partition_broadcast(reca[:], rec_a[:])
                        recb = normp.tile([128, SQT], BF16, tag="recb")
                        nc.gpsimd.partition_broadcast(recb[:], rec_b[:])
                        nc.vector.tensor_tensor(
                            out=attnT2[0:64, jsl], in0=unnorm[0:64, jsl],
                            in1=reca[0:64, :], op=mult)
                        nc.vector.tensor_tensor(
                            out=attnT2[64:128, jsl], in0=unnorm[64:128, jsl],
                            in1=recb[64:128, :], op=mult)

                        ag_in = dramp.tile([128, SQT], BF16, tag="ag_in")
                        nc.sync.dma_start(ag_in[:], attnT2[:, jsl])
                        ag_out = dramp.tile([2, 128, SQT], BF16,
                                            tag="ag_out")
                        nc.gpsimd.collective_compute(
                            "AllGather", mybir.AluOpType.bypass,
                            ins=[ag_in[:].opt()], outs=[ag_out[:].opt()],
                            replica_groups=groups)
                        agd[(hp, j)] = ag_out

                    # normalize/AllGather deferred into the next slice's
                    # block loop so the denorm ACT ops never gap the exps
                    pending_tail[0] = tail

            if pending_tail[0] is not None:
                pending_tail[0]()
                pending_tail[0] = None

            # --- drain: remaining pair23 Wo chunks -----------------------
            load_ag([(2, j) for j in range(NSQ - 2, NSQ)]
                    + [(3, j) for j in range(NSQ - 2, NSQ)])
            for j in range(NSQ - 2, NSQ):
                for t in range(SQT // 128):
                    emit_wo_quarter(2, 3, j, t, final=True)

    nc.compile()
    return nc


# ---------------------------------------------------------------------------
# Host-side sharding / unsharding
# ---------------------------------------------------------------------------

def _host_inputs(x, Wq, Wk, Wv, Wo, token_positions, n_cores, S):
    import ml_dtypes
    bf16 = ml_dtypes.bfloat16
    D = D_MODEL
    NC = D // 128
    NHP = 4

    # rope tables.  Partition layout within each head (64 partitions):
    # [e0..e15, o0..o15, e16..e31, o16..o31] -- the rotation partner sits
    # 16 partitions away inside the same 32-group, so the kernel's
    # stream_shuffle (a per-32-group lane shuffle) can realize the swap.
    pos = np.asarray(token_positions).astype(np.float32)  # (S,)
    i32 = np.arange(32, dtype=np.float32)
    inv_freq = ROPE_THETA ** (-i32 / 32.0)
    ang = pos[None, :] * inv_freq[:, None]              # (32, S)
    p = np.arange(128)
    pp = p % 64
    g, o = pp // 32, pp % 32
    freq_idx = 16 * g + (o % 16)                        # (128,)
    sign = np.where(o % 32 < 16, -1.0, 1.0)             # even slots: -sin
    cosT = np.cos(ang[freq_idx, :]).astype(bf16)        # (128, S)
    sinT = (sign[:, None] * np.sin(ang[freq_idx, :])).astype(bf16)

    # causal boundary strip mask (same pattern for every diagonal block
    # offset), duplicated for the two heads: valid iff pcol <= f
    pcol = np.arange(128)[:, None]
    f = np.arange(128)[None, :]
    strip = (pcol <= f)
    masks = np.tile(strip, (1, 2)).astype(bf16)         # (128, 256)

    # de-interleaving row permutation for q/k (see rope table comment)
    def qk_rows(grp):
        rows = []
        for h in range(8 * grp, 8 * grp + 8):
            rows += [h * DH + 2 * i for i in range(16)]
            rows += [h * DH + 2 * i + 1 for i in range(16)]
            rows += [h * DH + 2 * i for i in range(16, 32)]
            rows += [h * DH + 2 * i + 1 for i in range(16, 32)]
        return rows

    def wqk_layout(W, grp):
        # (D, 512) -> [128, NC, NHP, 128]
        t = W[qk_rows(grp), :].T.astype(bf16)
        return np.ascontiguousarray(
            t.reshape(NC, 128, NHP, 128).transpose(1, 0, 2, 3))

    def wv_layout(W, grp):
        t = W[512 * grp:512 * grp + 512, :].T.astype(bf16)
        return np.ascontiguousarray(
            t.reshape(NC, 128, NHP, 128).transpose(1, 0, 2, 3))

    def wo_layout(W, grp):
        t = W.T[:, 512 * grp:512 * grp + 512].astype(bf16)  # (D, 512)
        return np.ascontiguousarray(t.reshape(NC, 128, SQT).transpose(1, 0, 2))

    in_maps = []
    for c in range(n_cores):
        b, grp = c // 2, c % 2
        xb = np.ascontiguousarray(x[b].T).astype(bf16)  # (D, S)
        in_maps.append({
            "xT": np.ascontiguousarray(
                xb.reshape(NC, 128, S).transpose(1, 0, 2)),
            "wqT": wqk_layout(Wq, grp),
            "wkT": wqk_layout(Wk, grp),
            "wvT": wv_layout(Wv, grp),
            "woT": wo_layout(Wo, grp),
            "cosT": cosT,
            "sinT": sinT,
            "masks": masks.reshape(128, 2, 128),
        })
    return in_maps


def _assemble(results, n_cores, S):
    B = n_cores // 2
    full = np.empty((B, S, D_MODEL), dtype=np.float32)
    for c in range(n_cores):
        b, grp = c // 2, c % 2
        full[b, :, 512 * grp:512 * grp + 512] = results[c]["out"]
    return full


# ---------------------------------------------------------------------------
# Entry point
# ---------------------------------------------------------------------------

_NC_CACHE = {}


def _get_nc(n_cores, S):
    key = (n_cores, S)
    if key not in _NC_CACHE:
        _NC_CACHE[key] = build_kernel(n_cores, S)
    return _NC_CACHE[key]


def kernel(x, Wq, Wk, Wv, Wo, token_positions, _trace=False, _tmpdir=None):
    from concourse.bass_utils import run_bass_kernel_spmd

    x = np.asarray(x)
    B, S, D = x.shape
    n_cores = 2 * B
    nc = _get_nc(n_cores, S)
    in_maps = _host_inputs(np.asarray(x), np.asarray(Wq), np.asarray(Wk),
                           np.asarray(Wv), np.asarray(Wo),
                           np.asarray(token_positions), n_cores, S)
    res = run_bass_kernel_spmd(nc, in_maps, core_ids=list(range(n_cores)),
                               trace=_trace, tmpdir=_tmpdir)
    out = _assemble(res.results, n_cores, S)
    if _trace:
        return out, res
    return out
